# revision 44
# baseline (speedup 1.0000x reference)
"""ANT_Linear fused kernel for 8 TRN2 NeuronCores (raw Bass, manual sems).

out = fakequant(x) @ W.T + bias; per-128-group absmax scaling of x snapped to
the 15-level e2m1 ('flint') grid.  Data-parallel over tokens: 2048/core,
16 tiles of [128 tokens, 4096 features].

Math: with v = xs = x*(6/absmax), |v| <= 6 by construction.  Grid snap:
  y = veltkamp2(v) = c - (c - v), c = v*(2^22+1)   (round to 2 sig bits)
  h = (v + C) - C, C = 1.5*2^22                    (round to nearest 0.5)
  snap = h where |v| < 2 else y      (both are correct on 0.875..2.25)
|v|>=2 in ONE bitwise op: bits(v) & 0x40000000 (f32 exponent >= 128).

Per tile [128, 4096]:
  sync  : DMA x tile in (3-deep rotation)
  DVE   : absmax reduce + 6/amax + amax/6 (stats); mask=band(v); t=(v*K)-v;
          y=(v*K)-t in place over v; copy_predicated(h_sb <- y where mask)
          => q lands in h_sb, and the x buffer frees right at pc.
  GPSIMD: xs = x*rr (bcast, in place); dq = q*scale -> bf16
  ACT   : h = two Copy passes (+C, -C) -> h_sb; PSUM->SBUF copybacks;
          out copy + out DMA
  PE    : 32 transposes of dq k-blocks (identity mm) -> PSUM staging;
          64 matmuls (dqT.T @ WT) + 2 bias matmuls -> PSUM out
"""

import numpy as np
import ml_dtypes

N_CORES = 8
TOK = 4 * 4096
TPC = TOK // N_CORES    # 2048
K = 4096
M = 1024
GS = 128
G = K // GS             # 32
TT = 128
NT = TPC // TT          # 16

K_VELT = 4194305.0      # 2^22 + 1
C_HALF = 6291456.0      # 1.5 * 2^22

_CACHE = {}


def _build_bass(nrep=1):
    from contextlib import ExitStack

    import concourse.bass as bass
    import concourse.mybir as mybir

    dt = mybir.dt
    alu = mybir.AluOpType
    AF = mybir.ActivationFunctionType

    nc = bass.Bass()
    x_d = nc.declare_dram_parameter("x", [TPC, K], dt.float32, isOutput=False)
    wt_d = nc.declare_dram_parameter("wt", [K, M], dt.bfloat16, isOutput=False)
    b_d = nc.declare_dram_parameter("bias", [1, M], dt.bfloat16, isOutput=False)
    id_d = nc.declare_dram_parameter("ident", [128, 128], dt.bfloat16, isOutput=False)
    out_d = nc.declare_dram_parameter("out", [TPC, M], dt.float32, isOutput=True)

    x_t4 = x_d.rearrange("(n p) (g s) -> n p g s", p=TT, s=GS)   # [16,128,32,128]
    wt_t3 = wt_d.rearrange("(b p) m -> p b m", p=128)            # [128,32,1024]

    NXB = 3
    NG = nrep * NT

    ctx = ExitStack()
    with ctx:
        sb = lambda name, shape, d: ctx.enter_context(nc.sbuf_tensor(name, shape, d))
        ps = lambda name, shape, d: ctx.enter_context(nc.psum_tensor(name, shape, d))
        sem = lambda name: ctx.enter_context(nc.semaphore(name))

        wt_sb = sb("wt_sb", [128, G, M], dt.bfloat16)           # 8 MiB resident
        bias_sb = sb("bias_sb", [1, M], dt.bfloat16)
        id_sb = sb("id_sb", [128, 128], dt.bfloat16)
        ones_sb = sb("ones_sb", [1, TT], dt.bfloat16)

        x_sb = [sb(f"x_sb{k}", [TT, G, GS], dt.float32) for k in range(NXB)]
        t_sb = sb("t_sb", [TT, K], dt.float32)
        h_sb = [sb(f"h_sb{k}", [TT, K], dt.float32) for k in range(2)]
        m_sb = sb("m_sb", [TT, K], dt.int32)
        dq_sb = sb("dq_sb", [TT, G, GS], dt.bfloat16)
        dqt_sb = [sb(f"dqt_sb{k}", [128, G, TT], dt.bfloat16) for k in range(2)]
        o_sb = sb("o_sb", [TT, M], dt.float32)
        amax_sb = sb("amax_sb", [TT, G], dt.float32)
        rr_sb = [sb(f"rr_sb{k}", [TT, G], dt.float32) for k in range(2)]
        sc_sb = [sb(f"sc_sb{k}", [TT, G], dt.float32) for k in range(3)]

        stage_ps = [ps(f"stage_ps{k}", [128, 8, TT], dt.bfloat16) for k in range(2)]
        pout_ps = [ps(f"pout_ps{k}", [TT, M], dt.float32) for k in range(2)]

        sC = sem("sC")
        sX = sem("sX")     # x DMA in (+16/tile)
        sR = sem("sR")     # DVE stats done
        sA = sem("sA")     # gpsimd xs done
        sB = sem("sB")     # ACT h done
        sY = sem("sY")     # DVE pc done (q final in h_sb; x buffer free)
        sQ = sem("sQ")     # gpsimd dq done (h buffer free)
        sT = sem("sT")     # PE transpose chunk (+4/tile)
        sU = sem("sU")     # ACT copyback chunk (+4/tile)
        sM = sem("sM")     # PE matmuls done
        sO = sem("sO")     # ACT out-copy done
        sD = sem("sD")     # out DMA done (+16/tile)
        sV = sem("sV")     # ACT t' = v*2^22 done

        xflat = [x_sb[k].rearrange("p g s -> p (g s)") for k in range(NXB)]
        dqflat = dq_sb.rearrange("p g s -> p (g s)")

        with nc.Block() as block:

            @block.sync
            def _(eng):
                for i in range(NXB):
                    eng.dma_start(
                        out=x_sb[i][:, :, :], in_=x_t4[i]
                    ).then_inc(sX, 16)
                eng.dma_start(out=wt_sb[:, :, :], in_=wt_t3).then_inc(sC, 16)
                eng.dma_start(out=bias_sb[:, :], in_=b_d[:, :]).then_inc(sC, 16)
                eng.dma_start(out=id_sb[:, :], in_=id_d[:, :]).then_inc(sC, 16)
                for i in range(NXB, NG):
                    eng.wait_ge(sY, i - NXB + 1)
                    eng.dma_start(
                        out=x_sb[i % NXB][:, :, :], in_=x_t4[i % NT]
                    ).then_inc(sX, 16)

            @block.vector
            def _(eng):
                nc.vector.memset(ones_sb[:, :], 1.0)
                nc.vector.drain().then_inc(sC, 1)

                def stats(j):
                    eng.wait_ge(sX, 16 * (j + 1))
                    nc.vector.tensor_reduce(
                        out=amax_sb[:, :], in_=x_sb[j % NXB][:, :, :],
                        axis=mybir.AxisListType.X, op=alu.max,
                        apply_absolute_value=True,
                    )
                    nc.vector.drain()
                    if j >= 1:
                        eng.wait_ge(sA, j - 1)         # rr[j%2] free
                    if j >= 2:
                        eng.wait_ge(sQ, j - 2)         # sc[j%3] free
                    # scale = max(amax/6, tiny)  (one chained tensor_scalar)
                    nc.vector.tensor_scalar(
                        out=sc_sb[j % 3][:, :], in0=amax_sb[:, :],
                        scalar1=1.0 / 6.0, scalar2=1e-30,
                        op0=alu.mult, op1=alu.max,
                    )
                    nc.vector.drain()
                    nc.vector.reciprocal(out=rr_sb[j % 2][:, :], in_=sc_sb[j % 3][:, :])
                    nc.vector.drain().then_inc(sR, 1)

                stats(0)
                for i in range(NG):
                    if i + 1 < NG:
                        stats(i + 1)
                    eng.wait_ge(sA, i + 1)             # xs(i) done
                    # mask_big = bits(v) & 0x40000000: nonzero iff |v| >= 2
                    nc.vector.tensor_scalar(
                        out=m_sb[:, :],
                        in0=xflat[i % NXB][:, :].bitcast(mybir.dt.int32),
                        scalar1=0x40000000, scalar2=None, op0=alu.bitwise_and,
                    )
                    # t = v*2^22 (exact; equals the Veltkamp c-v up to the
                    # rounding already absorbed in c, and c-t is exact)
                    nc.vector.tensor_scalar_mul(
                        out=t_sb[:, :], in0=xflat[i % NXB][:, :], scalar1=4194304.0
                    )
                    nc.vector.drain()
                    # y = (v*K) - t   (in place over t; xs stays intact)
                    nc.vector.scalar_tensor_tensor(
                        out=t_sb[:, :], in0=xflat[i % NXB][:, :],
                        scalar=K_VELT, in1=t_sb[:, :],
                        op0=alu.mult, op1=alu.subtract,
                    )
                    nc.vector.drain()
                    # q = h, overwritten with y where |v| >= 2  (into h_sb)
                    eng.wait_ge(sB, i + 1)             # h(i) present in h_sb
                    nc.vector.copy_predicated(
                        out=h_sb[i % 2][:, :], mask=m_sb[:, :],
                        data=t_sb[:, :],
                    )
                    nc.vector.drain().then_inc(sY, 1)

            @block.gpsimd
            def _(eng):
                def mk_xs(j):
                    eng.wait_ge(sR, j + 1)
                    eng.wait_ge(sX, 16 * (j + 1))
                    r_b = rr_sb[j % 2][:, :].unsqueeze(2).broadcast_to((TT, G, GS))
                    nc.gpsimd.tensor_tensor(
                        out=x_sb[j % NXB][:, :, :], in0=x_sb[j % NXB][:, :, :],
                        in1=r_b[:, :, :], op=alu.mult,
                    )
                    nc.gpsimd.drain().then_inc(sA, 1)

                mk_xs(0)
                for i in range(NG):
                    if i + 1 < NG:
                        mk_xs(i + 1)
                    eng.wait_ge(sY, i + 1)             # q final in h_sb[i%2]
                    if i >= 1:
                        eng.wait_ge(sT, 4 * i)         # dq free (tp(i-1) done)
                    s_b = sc_sb[i % 3][:, :].unsqueeze(2).broadcast_to((TT, G, GS))
                    nc.gpsimd.tensor_tensor(
                        out=dq_sb[:, :, :],
                        in0=h_sb[i % 2].rearrange("p (g s) -> p g s", s=GS)[:, :, :],
                        in1=s_b[:, :, :], op=alu.mult,
                    )
                    nc.gpsimd.drain().then_inc(sQ, 1)

            @block.scalar
            def _(eng):
                def mk_h(j):
                    eng.wait_ge(sA, j + 1)
                    if j >= 2:
                        eng.wait_ge(sQ, j - 1)         # h[j%2] free (dq(j-2) done)
                    nc.scalar.activation(
                        out=h_sb[j % 2][:, :], in_=xflat[j % NXB][:, :],
                        func=AF.Copy, bias=C_HALF,
                    )
                    nc.scalar.drain()
                    nc.scalar.activation(
                        out=h_sb[j % 2][:, :], in_=h_sb[j % 2][:, :],
                        func=AF.Copy, bias=-C_HALF,
                    )
                    nc.scalar.drain().then_inc(sB, 1)

                def out_copy(j):
                    eng.wait_ge(sM, j + 1)
                    if j >= 1:
                        eng.wait_ge(sD, 16 * j)        # o_sb free
                    nc.scalar.activation(
                        out=o_sb[:, :], in_=pout_ps[j % 2][:, :], func=AF.Copy
                    )
                    nc.scalar.drain().then_inc(sO, 1)
                    eng.dma_start(
                        out=out_d[(j % NT) * TT:((j % NT) + 1) * TT, :],
                        in_=o_sb[:, :]
                    ).then_inc(sD, 16)

                mk_h(0)
                for i in range(NG):
                    if i + 1 < NG:
                        mk_h(i + 1)
                    if i >= 1:
                        out_copy(i - 1)
                    if i >= 1:
                        eng.wait_ge(sM, i)             # dqt[i%2] free
                    for c in range(4):
                        eng.wait_ge(sT, 4 * i + c + 1)
                        nc.scalar.activation(
                            out=dqt_sb[i % 2][:, c * 8:(c + 1) * 8, :],
                            in_=stage_ps[c % 2][:, :, :], func=AF.Copy,
                        )
                        nc.scalar.drain().then_inc(sU, 1)
                out_copy(NG - 1)

            @block.tensor
            def _(eng):
                eng.wait_ge(sC, 49)
                for i in range(NG):
                    eng.wait_ge(sQ, i + 1)
                    for c in range(4):
                        if 4 * i + c >= 2:
                            eng.wait_ge(sU, 4 * i + c - 1)   # stage[c%2] free
                        for j in range(8):
                            b = c * 8 + j
                            ins = nc.tensor.transpose(
                                stage_ps[c % 2][:, j, :], dq_sb[:, b, :],
                                id_sb[:, :],
                            )
                            if j == 7:
                                ins.then_inc(sT, 1)
                    if i >= 2:
                        eng.wait_ge(sO, i - 1)               # pout[i%2] free
                    for c in range(4):
                        eng.wait_ge(sU, 4 * i + c + 1)
                        for j in range(8):
                            b = c * 8 + j
                            for hf in range(2):
                                nc.tensor.matmul(
                                    pout_ps[i % 2][:, hf * 512:(hf + 1) * 512],
                                    lhsT=dqt_sb[i % 2][:, b, :],
                                    rhs=wt_sb[:, b, hf * 512:(hf + 1) * 512],
                                    start=(b == 0),
                                    stop=False,
                                )
                    for hf in range(2):
                        ins = nc.tensor.matmul(
                            pout_ps[i % 2][:, hf * 512:(hf + 1) * 512],
                            lhsT=ones_sb[:, :],
                            rhs=bias_sb[:, hf * 512:(hf + 1) * 512],
                            start=False,
                            stop=True,
                        )
                        if hf == 1:
                            ins.then_inc(sM, 1)

    return nc


def _get_nc(nrep=1):
    key = f"nc{nrep}"
    if key not in _CACHE:
        _CACHE[key] = _build_bass(nrep)
    return _CACHE[key]


def make_in_maps(x, weight, bias):
    x2 = np.ascontiguousarray(np.asarray(x, dtype=np.float32).reshape(TOK, K))
    wt = np.ascontiguousarray(np.asarray(weight, dtype=np.float32).T).astype(
        ml_dtypes.bfloat16
    )
    bias_b = np.asarray(bias, dtype=np.float32).reshape(1, M).astype(
        ml_dtypes.bfloat16
    )
    ident = np.eye(128, dtype=np.float32).astype(ml_dtypes.bfloat16)
    return [
        {"x": x2[i * TPC:(i + 1) * TPC], "wt": wt, "bias": bias_b, "ident": ident}
        for i in range(N_CORES)
    ]


def kernel(x, weight, bias, grid=None, **_ignored):
    from concourse.bass_utils import run_bass_kernel_spmd

    nc = _get_nc()
    in_maps = make_in_maps(x, weight, bias)
    res = run_bass_kernel_spmd(nc, in_maps, core_ids=list(range(N_CORES)))
    out = np.concatenate([res.results[i]["out"] for i in range(N_CORES)], axis=0)
    return out.reshape(4, 4096, M).astype(np.float32)
